# revision 10
# baseline (speedup 1.0000x reference)
"""DNC cell (nn_DNCCell) Trainium2 Bass kernel.

Sharding: data-parallel over batch B=16 across 8 cores (2 batches/core),
weights replicated. The updated link matrix is never materialized:
  link = (1-ww_i-ww_j)*L + ww_i*prec_j   (diag zeroed; input L has zero diag)
  bwd (L^T @ v): V-stationary streaming over natural L strips.
  fwd (L @ v):   folded through mem as (G^T L) with G=[mem | (1-ww)mem],
                 then a single [128,1024] transpose + small matmuls.
Allocation-by-sorted-usage is computed sort-free in log space:
  alloc_a = (1-u_a) * exp( sum_{b: u_b < u_a} ln u_b )
using a comparison matrix from tensor_scalar(is_gt)*ln(u) and ones-matvec.

Hardware constraints honored:
  - engine SBUF APs start only at partitions {0,32,64,96}; per-batch row
    extraction from [2,N] dual tiles goes through SBUF->SBUF DMA (exempt);
  - matmul output base partition must be 0/32/64;
  - PSUM pools reserve statically: "big" [128,1024] x3 (6 banks) +
    "med" <=2KB x2 (2 banks) = 8 banks exactly.
"""

import numpy as np

import concourse.bass as bass
import concourse.bacc as bacc
import concourse.tile as tile
from concourse import mybir
from concourse.masks import make_identity

F32 = mybir.dt.float32
F32R = mybir.dt.float32r
ALU = mybir.AluOpType
ACTF = mybir.ActivationFunctionType
AX = mybir.AxisListType

B, NCORES = 16, 8
BL = B // NCORES
A, M, R = 1024, 64, 4
AT = A // 128
I, H, O, IF = 256, 512, 256, 471
H4 = 4 * H
RM = R * M


def r32(ap):
    return ap.bitcast(F32R)


def build_nc():
    nc = bacc.Bacc("TRN2", target_bir_lowering=False, debug=False)
    d = {}
    d["x"] = nc.dram_tensor("x", [BL, I], F32, kind="ExternalInput")
    d["h_prev"] = nc.dram_tensor("h_prev", [BL, H], F32, kind="ExternalInput")
    d["c_prev"] = nc.dram_tensor("c_prev", [BL, H], F32, kind="ExternalInput")
    d["mem_prev"] = nc.dram_tensor("mem_prev", [BL, A, M], F32, kind="ExternalInput")
    d["rw_prev"] = nc.dram_tensor("rw_prev", [BL, R, A], F32, kind="ExternalInput")
    d["ww_prev"] = nc.dram_tensor("ww_prev", [BL, A], F32, kind="ExternalInput")
    d["usage_prev"] = nc.dram_tensor("usage_prev", [BL, A], F32, kind="ExternalInput")
    d["prec_prev"] = nc.dram_tensor("prec_prev", [BL, A], F32, kind="ExternalInput")
    d["link_prev"] = nc.dram_tensor("link_prev", [BL, A, A], F32, kind="ExternalInput")
    d["rv_prev"] = nc.dram_tensor("rv_prev", [BL, RM], F32, kind="ExternalInput")
    d["Wx"] = nc.dram_tensor("Wx", [I + RM, H4], F32, kind="ExternalInput")
    d["Wh"] = nc.dram_tensor("Wh", [H, H4], F32, kind="ExternalInput")
    d["b_lstm"] = nc.dram_tensor("b_lstm", [H4], F32, kind="ExternalInput")
    d["W_hid"] = nc.dram_tensor("W_hid", [H, O], F32, kind="ExternalInput")
    d["b_hid"] = nc.dram_tensor("b_hid", [O], F32, kind="ExternalInput")
    d["W_if"] = nc.dram_tensor("W_if", [H, IF], F32, kind="ExternalInput")
    d["b_if"] = nc.dram_tensor("b_if", [IF], F32, kind="ExternalInput")
    d["W_rd"] = nc.dram_tensor("W_rd", [RM, O], F32, kind="ExternalInput")
    d["b_rd"] = nc.dram_tensor("b_rd", [O], F32, kind="ExternalInput")
    d["out"] = nc.dram_tensor("out", [BL, O], F32, kind="ExternalOutput")

    with tile.TileContext(nc) as tc:
        _body(nc, tc, d)
    nc.finalize()
    return nc


def _body(nc, tc, d):
    import contextlib
    stack = contextlib.ExitStack()
    P = lambda name, bufs, space="SBUF": stack.enter_context(
        tc.tile_pool(name=name, bufs=bufs, space=space))

    pw = P("pw", 1)        # small weights + biases (unique tags)
    wxh = P("wxh", 4)      # LSTM weight blocks [128,512], rotating
    pp = P("pp", 1)        # persistents
    rows = P("rows", 7)    # [1,1024] transient rows, tag "row"
    d1 = P("d1", 6)        # [<=4,1024] transient duals, tag "d1"
    gg = P("gg", 7)        # [2,512] gate-phase tiles, tag "gg"
    lp = P("lp", 3)        # link strips [128,1024]
    ctp = P("ctp", 2)      # comparison chunks [128,1024]
    msh = P("msh", 1)      # mem/HG shared-tag tiles (serialize batches)
    psB = P("psB", 3, "PSUM")   # tag "big" [128,1024] -> 6 banks
    psM = P("psM", 2, "PSUM")   # tag "med" <=2KB -> 2 banks

    dma = nc.sync.dma_start
    _ctr = [0]

    def _nm(p):
        _ctr[0] += 1
        return f"{p}{_ctr[0]}"

    big = lambda shp=[128, A]: psB.tile(shp, F32, tag="big", name=_nm("big"))
    med = lambda shp: psM.tile(shp, F32, tag="med", name=_nm("med"))
    row = lambda: rows.tile([1, A], F32, tag="row", name=_nm("row"))
    dd = lambda shp=[2, A]: d1.tile(shp, F32, tag="d1", name=_nm("dd"))
    gt = lambda shp=[2, H]: gg.tile(shp, F32, tag="gg", name=_nm("gg"))

    # ---------------- constants ----------------
    ident = pp.tile([128, 128], F32)
    make_identity(nc, ident[:])
    ones_r = pp.tile([1, 128], F32)
    nc.vector.memset(ones_r[:], 1.0)
    ones_f = pp.tile([128, 4], F32)
    nc.vector.memset(ones_f[:], 1.0)
    ones_c = pp.tile([128, 1], F32R)
    nc.scalar.copy(ones_c[:], ones_f[:, 0:1])
    ones4r = pp.tile([1, 4], F32R)
    nc.scalar.copy(ones4r[:], ones_f[0:1, :])

    def transpose(dst_ap, src_ap, psum_shape):
        # PE transpose src [K, M] -> psum [M, K], copy to dst (SBUF)
        k = src_ap.shape[0]
        pt = med(list(psum_shape))
        nc.tensor.transpose(pt[:], src_ap, ident[0:k, 0:k])
        nc.scalar.copy(dst_ap, pt[:])

    # ACT table on this compiler supports one clean set {exp, ln, square,
    # copy, identity}; synthesize the rest from exp/ln.
    def _fs(ap):
        n = 1
        for s in ap.shape[1:]:
            n *= s
        return n

    def sigm(out_ap, in_ap, alloc):
        n = _fs(in_ap)
        t = alloc()[:, 0:n]
        nc.scalar.activation(t, in_ap, ACTF.Exp, scale=-1.0)
        nc.vector.tensor_scalar(t, t, 1.0, None, ALU.add)
        nc.vector.reciprocal(out_ap, t)

    def tanh_(out_ap, in_ap, alloc):
        n = _fs(in_ap)
        t = alloc()[:, 0:n]
        u = alloc()[:, 0:n]
        nc.scalar.activation(t, in_ap, ACTF.Exp, scale=-2.0)
        nc.vector.tensor_scalar(u, t, -1.0, 1.0, ALU.mult, ALU.add)
        nc.vector.tensor_scalar(t, t, 1.0, None, ALU.add)
        nc.vector.reciprocal(t, t)
        nc.vector.tensor_mul(out_ap, u, t)

    def sqrt_(out_ap, in_ap):
        nc.scalar.activation(out_ap, in_ap, ACTF.Ln)
        nc.scalar.activation(out_ap, out_ap, ACTF.Exp, scale=0.5)

    def stmp():
        return pp.tile([2, 64], F32, tag="stmp", name=_nm("stmp"))

    # ---------------- small weights + biases ----------------
    Wift = pw.tile([128, 4, IF + 1], F32R)
    Whidt = pw.tile([128, 4, O], F32R)
    Wrdt = pw.tile([128, 2, O], F32)
    for k in range(4):
        nc.gpsimd.dma_start(Wift[:, k, 0:IF], d["W_if"][k * 128:(k + 1) * 128, :])
        nc.vector.memset(Wift[:, k, IF:IF + 1].bitcast(F32), 0.0)
        nc.gpsimd.dma_start(Whidt[:, k, :], d["W_hid"][k * 128:(k + 1) * 128, :])
    for k in range(2):
        dma(Wrdt[:, k, :], d["W_rd"][k * 128:(k + 1) * 128, :])
    bif2 = pw.tile([2, IF], F32)
    bhid2 = pw.tile([2, O], F32)
    brd2 = pw.tile([2, O], F32)
    for b in range(2):
        dma(bif2[b:b + 1, :], d["b_if"][:].rearrange("(a n) -> a n", a=1))
        dma(bhid2[b:b + 1, :], d["b_hid"][:].rearrange("(a n) -> a n", a=1))
        dma(brd2[b:b + 1, :], d["b_rd"][:].rearrange("(a n) -> a n", a=1))

    # ---------------- state loads ----------------
    xc = pp.tile([2, I + RM], F32)
    dma(xc[:, 0:I], d["x"][:])
    dma(xc[:, I:I + RM], d["rv_prev"][:])
    hprev = pp.tile([2, H], F32)
    dma(hprev[:], d["h_prev"][:])
    cprev = pp.tile([2, H], F32)
    dma(cprev[:], d["c_prev"][:])
    wwd = dd()
    dma(wwd[:], d["ww_prev"][:])
    upd = dd()
    dma(upd[:], d["usage_prev"][:])
    precd = dd()
    dma(precd[:], d["prec_prev"][:])
    rwdual = pp.tile([2, R * A], F32)
    dma(rwdual[:], d["rw_prev"][:].rearrange("b r a -> b (r a)"))
    stack8 = pp.tile([8, A], F32)
    for b in range(2):
        dma(stack8[4 * b:4 * b + 4, :], d["rw_prev"][b])
    memP = [pp.tile([128, AT, M], F32, tag=f"memP{b}", name=f"memP{b}") for b in range(2)]
    for b in range(2):
        dma(memP[b][:], d["mem_prev"][b].rearrange("(c p) m -> p c m", p=128))

    # ---------------- controller LSTM (per 512-col round = one gate) ----
    xcT = pp.tile([128, 4, 2], F32R)
    hT0 = pp.tile([128, 4, 2], F32R)
    for c in range(4):
        transpose(xcT[:, c, :], xc[:, c * 128:(c + 1) * 128], (128, 2))
        transpose(hT0[:, c, :], hprev[:, c * 128:(c + 1) * 128], (128, 2))

    gates = []
    for ns in range(4):
        sl = slice(ns * 512, (ns + 1) * 512)
        zr = med([2, 512])
        for kc in range(8):
            wb = wxh.tile([128, 512], F32R, tag="wxh", name=_nm("wb"))
            src = d["Wx"] if kc < 4 else d["Wh"]
            nc.gpsimd.dma_start(wb[:], src[(kc % 4) * 128:(kc % 4 + 1) * 128, sl])
            lhs = xcT[:, kc, :] if kc < 4 else hT0[:, kc - 4, :]
            nc.tensor.matmul(zr[:], lhs, wb[:],
                             start=(kc == 0), stop=(kc == 7))
        bb = gt()
        for b in range(2):
            dma(bb[b:b + 1, :], d["b_lstm"][sl].rearrange("(a n) -> a n", a=1))
        zz = gt()
        nc.vector.tensor_add(zz[:], zr[:], bb[:])
        go = gt()
        if ns == 2:
            tanh_(go[:], zz[:], gt)
        else:
            sigm(go[:], zz[:], gt)
        gates.append(go)
    sgi, sgf, tng, sgo = gates

    t1 = gt()
    nc.vector.tensor_mul(t1[:], sgf[:], cprev[:])
    t2 = gt()
    nc.vector.tensor_mul(t2[:], sgi[:], tng[:])
    cst = gt()
    nc.vector.tensor_add(cst[:], t1[:], t2[:])
    tnc = gt()
    tanh_(tnc[:], cst[:], gt)
    hnew = pp.tile([2, H], F32)
    nc.vector.tensor_mul(hnew[:], sgo[:], tnc[:])

    hT = pp.tile([128, 4, 2], F32R)
    for c in range(4):
        transpose(hT[:, c, :], hnew[:, c * 128:(c + 1) * 128], (128, 2))

    ips = med([2, IF + 1])
    for kc in range(4):
        nc.tensor.matmul(ips[:], hT[:, kc, :], Wift[:, kc, :],
                         start=(kc == 0), stop=(kc == 3))
    ifc = pp.tile([2, IF], F32)
    nc.vector.tensor_add(ifc[:], ips[:, 0:IF], bif2[:])

    ohp = med([2, O])
    for kc in range(4):
        nc.tensor.matmul(ohp[:], hT[:, kc, :], Whidt[:, kc, :],
                         start=(kc == 0), stop=(kc == 3))
    oh = pp.tile([2, O], F32)
    nc.vector.tensor_add(oh[:], ohp[:], bhid2[:])

    # ---------------- interface pieces ----------------
    p0 = 0
    sl_kr = slice(p0, p0 + RM); p0 += RM
    sl_br = slice(p0, p0 + R); p0 += R
    sl_kw = slice(p0, p0 + M); p0 += M
    sl_bw = slice(p0, p0 + 1); p0 += 1
    sl_er = slice(p0, p0 + M); p0 += M
    sl_wv = slice(p0, p0 + M); p0 += M
    sl_fr = slice(p0, p0 + R); p0 += R
    sl_ga = slice(p0, p0 + 1); p0 += 1
    sl_gw = slice(p0, p0 + 1); p0 += 1
    sl_pi = slice(p0, p0 + 3 * R); p0 += 3 * R
    assert p0 == IF

    sm = pp.tile([2, 40], F32)   # small dual scalars: see slots below
    # slots: 0:4 betar | 4:8 knr | 8:9 knw | 9:10 betaw | 10:11 ga
    #        11:12 omga | 12:13 gw | 13:17 s1 | 17:21 s2 | 21:33 pi
    tb = stmp()
    nc.scalar.activation(tb[:, 0:4], ifc[:, sl_br], ACTF.Exp)
    nc.scalar.activation(tb[:, 4:5], ifc[:, sl_bw], ACTF.Exp)
    nc.vector.tensor_scalar(tb[:, 0:5], tb[:, 0:5], 1.0, None, ALU.add)
    nc.scalar.activation(tb[:, 0:5], tb[:, 0:5], ACTF.Ln)
    nc.vector.tensor_scalar(tb[:, 0:5], tb[:, 0:5], 1.0, None, ALU.add)
    nc.vector.tensor_copy(sm[:, 0:4], tb[:, 0:4])
    nc.vector.tensor_copy(sm[:, 9:10], tb[:, 4:5])
    erase = pp.tile([2, M], F32)
    sigm(erase[:], ifc[:, sl_er], stmp)
    free4 = pp.tile([2, R], F32)
    sigm(free4[:], ifc[:, sl_fr], stmp)
    sigm(sm[:, 10:11], ifc[:, sl_ga], stmp)
    nc.vector.tensor_scalar(sm[:, 11:12], sm[:, 10:11], -1.0, 1.0, ALU.mult, ALU.add)
    sigm(sm[:, 12:13], ifc[:, sl_gw], stmp)

    piv = ifc[:, sl_pi].rearrange("b (r m) -> b r m", r=R)
    pimax = pp.tile([2, R], F32)
    nc.vector.reduce_max(pimax[:], piv, axis=AX.X)
    pish = pp.tile([2, R, 3], F32)
    nc.vector.tensor_tensor(pish[:], piv,
                            pimax[:, :, None].broadcast_to((2, R, 3)), ALU.subtract)
    nc.scalar.activation(pish[:], pish[:], ACTF.Exp)
    pisum = pp.tile([2, R], F32)
    nc.vector.reduce_sum(pisum[:], pish[:], axis=AX.X)
    nc.vector.reciprocal(pisum[:], pisum[:])
    nc.vector.tensor_tensor(sm[:, 21:33].rearrange("b (r m) -> b r m", r=R),
                            pish[:], pisum[:, :, None].broadcast_to((2, R, 3)),
                            ALU.mult)

    # key norms
    tk = dd([2, RM])
    nc.vector.tensor_tensor(tk[:], ifc[:, sl_kr], ifc[:, sl_kr], ALU.mult)
    nc.vector.reduce_sum(sm[:, 4:8], tk[:].rearrange("b (r m) -> b r m", r=R),
                         axis=AX.X)
    sqrt_(sm[:, 4:8], sm[:, 4:8])
    tw = dd([2, M])
    nc.vector.tensor_tensor(tw[:], ifc[:, sl_kw], ifc[:, sl_kw], ALU.mult)
    nc.vector.reduce_sum(sm[:, 8:9], tw[:], axis=AX.X)
    sqrt_(sm[:, 8:9], sm[:, 8:9])

    # transposed keys kmat[64, 5, 2] (slots 0-3 k_r, 4 k_w)
    kmat = pp.tile([64, 5, 2], F32R)
    for j in range(4):
        transpose(kmat[:, j, :], ifc[:, sl_kr.start + j * M: sl_kr.start + (j + 1) * M],
                  (M, 2))
    transpose(kmat[:, 4, :], ifc[:, sl_kw], (M, 2))

    # scalar stack transpose: scs[2,64] (0:4 betar, 32:36 knr) -> scT[64,2]
    scs = pp.tile([2, 64], F32)
    nc.vector.tensor_copy(scs[:, 0:4], sm[:, 0:4])
    nc.vector.tensor_copy(scs[:, 32:36], sm[:, 4:8])
    scT = pp.tile([64, 2], F32)
    transpose(scT[:], scs[:], (64, 2))

    # ---------------- usage ----------------
    psi = dd()
    tmp_r = dd()
    for r in range(4):
        sl = slice(r * A, (r + 1) * A)
        dst = psi if r == 0 else tmp_r
        nc.vector.tensor_tensor(dst[:], rwdual[:, sl],
                                free4[:, r:r + 1].broadcast_to((2, A)), ALU.mult)
        nc.vector.tensor_scalar(dst[:], dst[:], -1.0, 1.0, ALU.mult, ALU.add)
        if r > 0:
            nc.vector.tensor_mul(psi[:], psi[:], tmp_r[:])
    # s1 = sum_a prec*rw per head
    for r in range(4):
        sl = slice(r * A, (r + 1) * A)
        nc.vector.tensor_tensor(tmp_r[:], rwdual[:, sl], precd[:], ALU.mult)
        nc.vector.reduce_sum(sm[:, 13 + r:14 + r], tmp_r[:], axis=AX.X)

    u1 = dd()
    nc.vector.tensor_mul(u1[:], upd[:], wwd[:])
    u2 = dd()
    nc.vector.tensor_add(u2[:], upd[:], wwd[:])
    usage = pp.tile([2, A], F32)
    nc.vector.tensor_sub(usage[:], u2[:], u1[:])
    nc.vector.tensor_mul(usage[:], usage[:], psi[:])
    nc.vector.tensor_scalar(usage[:], usage[:], 1e-38, None, ALU.max)

    # transpose usage+prec -> sptT[128, AT, 4] (0:2 uT, 2:4 precT)
    stack4 = dd([4, A])
    dma(stack4[0:2, :], usage[:])
    dma(stack4[2:4, :], precd[:])
    sptT = pp.tile([128, AT, 4], F32)
    for c in range(AT):
        transpose(sptT[:, c, :], stack4[:, c * 128:(c + 1) * 128], (128, 4))
    lgT = pp.tile([128, AT, 2], F32)
    nc.scalar.activation(lgT[:], sptT[:, :, 0:2], ACTF.Ln)

    urow = []
    for b in range(2):
        t = rows.tile([1, A], F32, tag="row", name=_nm("urow"))
        dma(t[:], usage[b:b + 1, :])
        urow.append(t)

    # per-batch scalar rows: b0 reads sm[0:1,:] directly; b1 via DMA copy
    smrow1 = pp.tile([1, 40], F32)
    dma(smrow1[:], sm[1:2, :])
    smrow = [sm[0:1, :], smrow1[:]]

    # ---------------- per-batch alloc + c_w -> ww ----------------
    ww_rows = []
    for b in range(2):
        sr = smrow[b]
        ubcP = big()
        for ns in range(2):
            sl = slice(ns * 512, (ns + 1) * 512)
            nc.tensor.matmul(ubcP[:, sl], ones_r[:], urow[b][:, sl],
                             start=True, stop=True)
        Sb = big([1, A])
        for c in range(AT):
            ctw = ctp.tile([128, A], F32R, tag="ct", name=_nm("ct"))
            nc.vector.tensor_scalar(ctw[:], ubcP[:], sptT[:, c, b:b + 1],
                                    lgT[:, c, b:b + 1], ALU.is_gt, ALU.mult)
            for ns in range(2):
                sl = slice(ns * 512, (ns + 1) * 512)
                nc.tensor.matmul(Sb[:, sl], ones_c[:], ctw[:, sl],
                                 start=(c == 0), stop=(c == AT - 1))
        aloc = row()
        nc.scalar.activation(aloc[:], Sb[:], ACTF.Exp)
        omu = row()
        nc.vector.tensor_scalar(omu[:], urow[b][:], -1.0, 1.0, ALU.mult, ALU.add)
        nc.vector.tensor_mul(aloc[:], aloc[:], omu[:])

        # memT_prev + squares
        memTp = msh.tile([64, AT, 128], F32R, tag="memTp")
        for c in range(AT):
            transpose(memTp[:, c, :], memP[b][:, c, :], (64, 128))
        mtp = memTp[:].rearrange("p c m -> p (c m)")
        sq = msh.tile([64, A], F32R, tag="sq")
        nc.scalar.activation(sq[:], mtp, ACTF.Square)
        dotw = big([1, A])
        mn2 = big([1, A])
        for ns in range(2):
            sl = slice(ns * 512, (ns + 1) * 512)
            nc.tensor.matmul(dotw[:, sl], kmat[:, 4, b:b + 1], mtp[:, sl],
                             start=True, stop=True)
            nc.tensor.matmul(mn2[:, sl], ones_c[0:64, :], sq[:, sl],
                             start=True, stop=True)
        den = row()
        sqrt_(den[:], mn2[:])
        nc.vector.tensor_scalar(den[:], den[:], sr[:, 8:9], 1e-6, ALU.mult, ALU.add)
        nc.vector.reciprocal(den[:], den[:])
        cosw = row()
        nc.vector.tensor_tensor(cosw[:], dotw[:], den[:], ALU.mult)
        mxw = pp.tile([1, 1], F32, tag="mxw")
        nc.vector.reduce_max(mxw[:], cosw[:], axis=AX.X)
        nc.vector.tensor_scalar(mxw[:], mxw[:], sr[:, 9:10], -1.0, ALU.mult, ALU.mult)
        ew = row()
        sw = pp.tile([1, 1], F32, tag="sw")
        nc.scalar.activation(ew[:], cosw[:], ACTF.Exp, bias=mxw[:],
                             scale=sr[:, 9:10], accum_out=sw[:])
        nc.vector.reciprocal(sw[:], sw[:])
        nc.vector.tensor_scalar(ew[:], ew[:], sw[:], None, ALU.mult)   # c_w

        wwr = rows.tile([1, A], F32, tag="row", name=_nm("wwr"))
        nc.vector.tensor_scalar(aloc[:], aloc[:], sr[:, 10:11], None, ALU.mult)
        nc.vector.scalar_tensor_tensor(wwr[:], ew[:], sr[:, 11:12], aloc[:],
                                       ALU.mult, ALU.add)
        nc.vector.tensor_scalar(wwr[:], wwr[:], sr[:, 12:13], None, ALU.mult)
        ww_rows.append(wwr)

    # ---------------- ww transposes + derived ----------------
    wwst = pp.tile([2, A], F32)
    for b in range(2):
        dma(wwst[b:b + 1, :], ww_rows[b][:])
    wwT = pp.tile([128, AT, 2], F32)
    for c in range(AT):
        transpose(wwT[:, c, :], wwst[:, c * 128:(c + 1) * 128], (128, 2))
    negwwT = pp.tile([128, AT, 2], F32)
    nc.vector.tensor_scalar(negwwT[:], wwT[:], -1.0, None, ALU.mult)
    omwT = pp.tile([128, AT, 2], F32)
    nc.vector.tensor_scalar(omwT[:], wwT[:], -1.0, 1.0, ALU.mult, ALU.add)
    negwpT = pp.tile([128, AT, 2], F32)
    nc.vector.scalar_tensor_tensor(negwpT[:], sptT[:, :, 2:4], -1.0, wwT[:],
                                   ALU.mult, ALU.mult)

    # s2 = sum_a ww*rw (new ww)
    tmp2 = dd()
    for r in range(4):
        sl = slice(r * A, (r + 1) * A)
        nc.vector.tensor_tensor(tmp2[:], rwdual[:, sl], wwst[:], ALU.mult)
        nc.vector.reduce_sum(sm[:, 17 + r:18 + r], tmp2[:], axis=AX.X)
    sm2row1 = pp.tile([1, 40], F32)
    dma(sm2row1[:], sm[1:2, :])
    smrow2 = [sm[0:1, :], sm2row1[:]]

    # ---------------- V = [rw | -ww*rw] transposed ----------------
    Vbig = pp.tile([128, AT, 16], F32R)
    for c in range(AT):
        pt = med([128, 8])
        nc.tensor.transpose(pt[:], stack8[:, c * 128:(c + 1) * 128], ident[0:8, 0:8])
        nc.scalar.copy(Vbig[:, c, 0:4], pt[:, 0:4])
        nc.scalar.copy(Vbig[:, c, 8:12], pt[:, 4:8])
    for b in range(2):
        nc.vector.tensor_tensor(
            Vbig[:, :, 8 * b + 4:8 * b + 8], Vbig[:, :, 8 * b:8 * b + 4],
            negwwT[:, :, b:b + 1].broadcast_to((128, AT, 4)), ALU.mult)
    wprA = [pp.tile([128, AT, 4], F32, tag=f"wprA{b}", name=f"wprA{b}") for b in range(2)]
    for b in range(2):
        nc.vector.tensor_tensor(
            wprA[b][:], Vbig[:, :, 8 * b:8 * b + 4],
            negwpT[:, :, b:b + 1].broadcast_to((128, AT, 4)), ALU.mult)

    # ---------------- per-batch heavy phase ----------------
    rvS = pp.tile([128, 2, 2], F32)
    for b in range(2):
        sr = smrow2[b]
        if b == 0:
            erow, wvrow = erase[0:1, :], ifc[0:1, sl_wv]
        else:
            et = pp.tile([1, M], F32, tag="erow")
            dma(et[:], erase[1:2, :])
            erow = et[:]
            wt_ = pp.tile([1, M], F32, tag="wvrow")
            dma(wt_[:], ifc[1:2, sl_wv])
            wvrow = wt_[:]
        eRep = med([128, M])
        nc.tensor.matmul(eRep[:], ones_r[:], erow, start=True, stop=True)
        wvRep = med([128, M])
        nc.tensor.matmul(wvRep[:], ones_r[:], wvrow, start=True, stop=True)

        # mem update (natural layout)
        memN = pp.tile([128, AT, M], F32, tag=f"memN{b}")
        wwb = wwT[:, :, b:b + 1].broadcast_to((128, AT, M))
        tA = msh.tile([128, AT, M], F32, tag="memu1")
        nc.vector.tensor_tensor(
            tA[:], eRep[:, None, :].broadcast_to((128, AT, M)), wwb, ALU.mult)
        tB = msh.tile([128, AT, M], F32, tag="memu2")
        nc.vector.tensor_mul(tB[:], memP[b][:], tA[:])
        nc.vector.tensor_sub(tB[:], memP[b][:], tB[:])
        nc.vector.tensor_tensor(
            tA[:], wvRep[:, None, :].broadcast_to((128, AT, M)), wwb, ALU.mult)
        nc.vector.tensor_add(memN[:], tB[:], tA[:])

        # memT new + squares
        memTn = msh.tile([64, AT, 128], F32R, tag="memTn")
        for c in range(AT):
            transpose(memTn[:, c, :], memN[:, c, :], (64, 128))
        mtn = memTn[:].rearrange("p c m -> p (c m)")
        sqn = msh.tile([64, A], F32R, tag="sq")
        nc.scalar.activation(sqn[:], mtn, ACTF.Square)

        # c_r
        drmn = big([4, A])
        mn2n = big([1, A])
        for ns in range(2):
            sl = slice(ns * 512, (ns + 1) * 512)
            nc.tensor.matmul(drmn[:, sl], kmat[:, 0:4, b], mtn[:, sl],
                             start=True, stop=True)
            nc.tensor.matmul(mn2n[:, sl], ones_c[0:64, :], sqn[:, sl],
                             start=True, stop=True)
        mnn = pp.tile([1, A], F32R, tag="mnn")
        sqrt_(mnn[:], mn2n[:])
        mnR = big([4, A])
        for ns in range(2):
            sl = slice(ns * 512, (ns + 1) * 512)
            nc.tensor.matmul(mnR[:, sl], ones4r[:], mnn[:, sl],
                             start=True, stop=True)
        denr = dd([4, A])
        nc.vector.tensor_scalar(denr[:], mnR[:], scT[32:36, b:b + 1], 1e-6,
                                ALU.mult, ALU.add)
        nc.vector.reciprocal(denr[:], denr[:])
        cosr = dd([4, A])
        nc.vector.tensor_tensor(cosr[:], drmn[:], denr[:], ALU.mult)
        mxr = pp.tile([4, 1], F32, tag="mxr")
        nc.vector.reduce_max(mxr[:], cosr[:], axis=AX.X)
        nc.vector.tensor_scalar(mxr[:], mxr[:], scT[0:4, b:b + 1], -1.0,
                                ALU.mult, ALU.mult)
        er_ = dd([4, A])
        sr_ = pp.tile([4, 1], F32, tag="sr_")
        nc.scalar.activation(er_[:], cosr[:], ACTF.Exp, bias=mxr[:],
                             scale=scT[0:4, b:b + 1], accum_out=sr_[:])
        nc.vector.reciprocal(sr_[:], sr_[:])
        nc.vector.tensor_scalar(er_[:], er_[:], sr_[:], None, ALU.mult)  # c_r
        crT = pp.tile([128, AT, 4], F32, tag="crT")
        for c in range(AT):
            transpose(crT[:, c, :], er_[:, c * 128:(c + 1) * 128], (128, 4))

        # G = [mem | (1-ww)mem]
        Gbig = msh.tile([128, AT, 128], F32R, tag="Gbig")
        nc.scalar.copy(Gbig[:, :, 0:M], memN[:])
        nc.vector.tensor_tensor(
            Gbig[:, :, M:128], memN[:],
            omwT[:, :, b:b + 1].broadcast_to((128, AT, M)), ALU.mult)

        # stream link strips: bwd (V-stationary) + HG (G-stationary)
        bwdY = big([8, A])
        HGp = big()
        for c in range(AT):
            Ls = lp.tile([128, A], F32R, tag="L", name=_nm("L"))
            nc.gpsimd.dma_start(Ls[:], d["link_prev"][b, c * 128:(c + 1) * 128, :])
            for ns in range(2):
                sl = slice(ns * 512, (ns + 1) * 512)
                nc.tensor.matmul(bwdY[:, sl], Vbig[:, c, 8 * b:8 * b + 8],
                                 Ls[:, sl], start=(c == 0), stop=(c == AT - 1))
                nc.tensor.matmul(HGp[:, sl], Gbig[:, c, :], Ls[:, sl],
                                 start=(c == 0), stop=(c == AT - 1))

        # mw = mem^T ww as row [1, 64]
        mwP = med([M, 1])
        for c in range(AT):
            nc.tensor.matmul(mwP[:], memN[:, c, :], wwT[:, c, b:b + 1],
                             start=(c == 0), stop=(c == AT - 1))
        mwc = pp.tile([M, 1], F32, tag="mwc")
        nc.scalar.copy(mwc[:], mwP[:])
        mwrow = pp.tile([1, M], F32, tag="mwrow")
        transpose(mwrow[:], mwc[:], (1, M))

        # bwd epilogue
        bwdS = msh.tile([8, A], F32, tag="bwdS")
        nc.scalar.copy(bwdS[:], bwdY[:])
        bwdT = pp.tile([128, AT, 8], F32, tag="bwdT")
        for c in range(AT):
            transpose(bwdT[:, c, :], bwdS[:, c * 128:(c + 1) * 128], (128, 8))
        s2Rep = med([128, R])
        nc.tensor.matmul(s2Rep[:], ones_r[:], sr[:, 17:21], start=True, stop=True)
        bwdW = pp.tile([128, AT, 4], F32, tag="bwdW")
        nc.vector.tensor_tensor(bwdW[:], bwdT[:, :, 0:4],
                                omwT[:, :, b:b + 1].broadcast_to((128, AT, 4)),
                                ALU.mult)
        nc.vector.tensor_add(bwdW[:], bwdW[:], bwdT[:, :, 4:8])
        tpe = pp.tile([128, AT, 4], F32, tag="tpe")
        nc.vector.tensor_tensor(
            tpe[:], s2Rep[:, None, :].broadcast_to((128, AT, 4)),
            sptT[:, :, 2 + b:3 + b].broadcast_to((128, AT, 4)), ALU.mult)
        nc.vector.tensor_add(bwdW[:], bwdW[:], tpe[:])
        nc.vector.tensor_add(bwdW[:], bwdW[:], wprA[b][:])

        # HG transpose
        HGs = msh.tile([128, A], F32, tag="HGs")
        nc.scalar.copy(HGs[:], HGp[:])
        HGT = msh.tile([128, AT, 128], F32R, tag="HGT")
        for c in range(AT):
            transpose(HGT[:, c, :], HGs[:, c * 128:(c + 1) * 128], (128, 128))

        # rv matvecs into one [64, 12] psum (cols 0:4 bwd, 4:8 c, 8:12 fwd)
        rv3 = med([M, 12])
        for c in range(AT):
            nc.tensor.matmul(rv3[:, 0:4], memN[:, c, :], bwdW[:, c, :],
                             start=(c == 0), stop=(c == AT - 1))
        for c in range(AT):
            nc.tensor.matmul(rv3[:, 4:8], memN[:, c, :], crT[:, c, :],
                             start=(c == 0), stop=(c == AT - 1))
        nmm = 3 * AT + 1
        k = 0
        for c in range(AT):
            nc.tensor.matmul(rv3[:, 8:12], HGT[:, c, M:128],
                             Vbig[:, c, 8 * b:8 * b + 4],
                             start=(k == 0), stop=(k == nmm - 1)); k += 1
            nc.tensor.matmul(rv3[:, 8:12], HGT[:, c, 0:M],
                             Vbig[:, c, 8 * b + 4:8 * b + 8],
                             start=(k == 0), stop=(k == nmm - 1)); k += 1
            nc.tensor.matmul(rv3[:, 8:12], memN[:, c, :], wprA[b][:, c, :],
                             start=(k == 0), stop=(k == nmm - 1)); k += 1
        nc.tensor.matmul(rv3[:, 8:12], mwrow[:], sr[:, 13:17],
                         start=(k == 0), stop=(k == nmm - 1)); k += 1

        # pi-replicated + combine into rvS (rm-chunk layout)
        piRp = med([128, 12])
        nc.tensor.matmul(piRp[:], ones_r[:], sr[:, 21:33], start=True, stop=True)
        piR = pp.tile([128, 12], F32, tag="piR")
        nc.scalar.copy(piR[:], piRp[:])
        for h in range(2):
            po = slice(h * 64, (h + 1) * 64)
            t0 = pp.tile([M, 2], F32, tag="rvt0")
            t1_ = pp.tile([M, 2], F32, tag="rvt1")
            nc.vector.tensor_tensor(t0[:], rv3[:, h:h + 3:2],
                                    piR[po, h * 3 + 0::6], ALU.mult)
            nc.vector.tensor_tensor(t1_[:], rv3[:, 4 + h:4 + h + 3:2],
                                    piR[po, h * 3 + 1::6], ALU.mult)
            nc.vector.tensor_add(t0[:], t0[:], t1_[:])
            nc.vector.tensor_tensor(t1_[:], rv3[:, 8 + h:8 + h + 3:2],
                                    piR[po, h * 3 + 2::6], ALU.mult)
            nc.vector.tensor_add(rvS[po, :, b], t0[:], t1_[:])

    # ---------------- final projection ----------------
    finp = med([2, O])
    for kc in range(2):
        nc.tensor.matmul(finp[:], rvS[:, kc, :], Wrdt[:, kc, :],
                         start=(kc == 0), stop=(kc == 1))
    fin = pp.tile([2, O], F32)
    nc.vector.tensor_add(fin[:], oh[:], finp[:])
    nc.vector.tensor_add(fin[:], fin[:], brd2[:])
    dma(d["out"][:], fin[:])

    stack.close()


_NC_CACHE = None


def make_in_maps(inputs):
    f = lambda v: np.ascontiguousarray(np.asarray(v, dtype=np.float32))
    full = {k: f(v) for k, v in inputs.items()}
    per_batch = ["x", "h_prev", "c_prev", "mem_prev", "rw_prev", "ww_prev",
                 "usage_prev", "prec_prev", "link_prev", "rv_prev"]
    shared = ["Wx", "Wh", "b_lstm", "W_hid", "b_hid", "W_if", "b_if",
              "W_rd", "b_rd"]
    in_maps = []
    for k in range(NCORES):
        m = {n: np.ascontiguousarray(full[n][k * BL:(k + 1) * BL])
             for n in per_batch}
        for n in shared:
            m[n] = full[n]
        in_maps.append(m)
    return in_maps


def run_bass(inputs, **kw):
    global _NC_CACHE
    from concourse.bass_utils import run_bass_kernel_spmd

    if _NC_CACHE is None:
        _NC_CACHE = build_nc()
    return run_bass_kernel_spmd(_NC_CACHE, make_in_maps(inputs),
                                list(range(NCORES)), **kw)


def kernel(**inputs):
    res = run_bass(inputs)
    return np.concatenate([res.results[k]["out"] for k in range(NCORES)], axis=0)
